# revision 18
# baseline (speedup 1.0000x reference)
"""Trainium2 Bass kernel for CustomMLP: out = GELU(x@W1+b1)@W2 + b2.

x: (4, 2048, 1024) f32, W1: (1024, 4096), b1: (4096,), W2: (4096, 1024),
b2: (1024,). Data-parallel over the 8192 flattened rows: each of the 8
NeuronCores handles 1024 rows with fully replicated weights (no
collectives).

Matmul operands are bf16 (host-cast); PSUM accumulation is fp32 and both
bias adds run in fp32 on the activation engine, so end-to-end rel err is
~3e-3 (gate 2e-2). bf16 halves DMA traffic and SBUF footprint vs the
fp32r version, which removes the DMA-induced PE stalls.

Per-core layout (everything transposed so both matmuls contract on the
partition axis with no on-chip transposes):
  xT   [1024(e), 1024(m)]           = x_shard^T
  hT   [h, m] computed on chip      (GELU applied on PSUM eviction)
  outT [1024(e2), 1024(m)]          host transposes back

matmul1: psum[h_blk, m] += w1[e_blk, h_blk].T @ xT[e_blk, m]
matmul2: psum[e2_blk, m] += w2[h_blk, e2_blk].T @ hT[h_blk, m]

Perf structure:
- A short warm-up of dummy matmuls on a memset tile runs while the first
  DMAs land, so the PE p-state ramp (0.65->2.4GHz over ~3us of busy time)
  is paid on dummy work instead of real matmuls.
- DMAs are batched (w1 in 4-block slabs, x in halves) to amortize the
  ~600ns/instruction SP sequencer enqueue cost that otherwise delays the
  first transfers.
- w2 is preloaded in full into SBUF during matmul-1 so matmul-2 never
  waits on DMA.
"""
import ml_dtypes
import numpy as np

import concourse.bass as bass
import concourse.mybir as mybir
import concourse.tile as tile
from concourse import bacc
from concourse.bass_utils import run_bass_kernel_spmd

P = 128
N_CORES = 8

F32 = mybir.dt.float32
BF16 = mybir.dt.bfloat16
GELU = mybir.ActivationFunctionType.Gelu
IDENT = mybir.ActivationFunctionType.Identity


def build_nc(M=1024, E=1024, H=4096, E2=1024, mm_dtype=BF16, act=GELU,
             warmup=10):
    """Build + compile the per-core program. M/E/H/E2 parameterized so a
    scaled-down version can run in CoreSim."""
    EB, HB, E2B = E // P, H // P, E2 // P
    MH = max(1, M // 512)  # m halves (moving-dim chunks of <=512)
    MS = M // MH           # moving chunk size
    W1G = min(4, HB)       # w1 h-blocks per DMA slab
    NS1 = HB // W1G        # number of w1 slabs

    mmdt = mm_dtype
    nc = bacc.Bacc(None, target_bir_lowering=False)
    xT_d = nc.declare_dram_parameter("xT", [E, M], mmdt, isOutput=False)
    w1_d = nc.declare_dram_parameter("w1p", [HB, P, EB, P], mmdt, isOutput=False)
    b1_d = nc.declare_dram_parameter("b1p", [P, HB], F32, isOutput=False)
    w2_d = nc.declare_dram_parameter("w2p", [E2B, P, HB, P], mmdt, isOutput=False)
    b2_d = nc.declare_dram_parameter("b2p", [P, E2B], F32, isOutput=False)
    out_d = nc.declare_dram_parameter("outT", [E2B, P, M], mmdt, isOutput=True)

    xT_v = xT_d.rearrange("(eb p) m -> p eb m", p=P)
    w1_v = w1_d.rearrange("h p e q -> p h e q")

    with tile.TileContext(nc) as tc:
        with (
            tc.tile_pool(name="const", bufs=1) as cpool,
            tc.tile_pool(name="xp", bufs=1) as xpool,
            tc.tile_pool(name="hp", bufs=1) as hpool,
            tc.tile_pool(name="w1p", bufs=3) as w1pool,
            tc.tile_pool(name="w2p", bufs=1) as w2pool,
            tc.tile_pool(name="op", bufs=2) as opool,
            tc.tile_pool(name="ps1", bufs=4, space="PSUM") as psum1,
            tc.tile_pool(name="ps2", bufs=3, space="PSUM") as psum2,
            tc.tile_pool(name="psw", bufs=1, space="PSUM") as psumw,
        ):
            # ---- PE warm-up: dummy matmuls on a zeroed tile while the
            # first real DMAs are in flight. Output PSUM is never read.
            if warmup:
                warm_sb = cpool.tile([P, 2 * P], mmdt, name="warm")
                nc.gpsimd.memset(warm_sb[:], 0.0)
                ps_w = psumw.tile([P, 2 * P], F32, name="psw")
                for _ in range(warmup):
                    nc.tensor.matmul(
                        ps_w[:], lhsT=warm_sb[:, 0:P], rhs=warm_sb[:],
                        start=True, stop=True,
                    )

            # ---- DMA emission. Two HWDGE queues run in parallel:
            #   SP queue (nc.sync):    w1 block0, w1 blocks 1-3, w1 slabs,
            #                          w2 slabs, out stores
            #   ACT queue (nc.scalar): x half0, b1, x half1, b2
            # so the x stream and the w1 stream transfer concurrently and
            # the first mm1 group's inputs land ~1us sooner.
            w1_tiles = {}
            w1_tiles[0] = w1pool.tile([P, W1G, EB, P], mmdt, name="w1t")
            nc.sync.dma_start(out=w1_tiles[0][:, 0:1], in_=w1_v[:, 0:1])

            xT_sb = xpool.tile([P, EB, M], mmdt, name="xT")
            nc.scalar.dma_start(out=xT_sb[:, :, 0:MS], in_=xT_v[:, :, 0:MS])

            b1_sb = cpool.tile([P, HB], F32, name="b1s")
            b2_sb = cpool.tile([P, E2B], F32, name="b2s")
            nc.scalar.dma_start(out=b1_sb[:], in_=b1_d[:])

            if W1G > 1:
                nc.sync.dma_start(out=w1_tiles[0][:, 1:W1G], in_=w1_v[:, 1:W1G])

            for mh in range(1, MH):
                ms = slice(mh * MS, (mh + 1) * MS)
                nc.scalar.dma_start(out=xT_sb[:, :, ms], in_=xT_v[:, :, ms])
            nc.scalar.dma_start(out=b2_sb[:], in_=b2_d[:])
            if NS1 > 1:
                w1_tiles[1] = w1pool.tile([P, W1G, EB, P], mmdt, name="w1t")
                nc.sync.dma_start(out=w1_tiles[1][:], in_=w1_v[:, W1G : 2 * W1G])

            hT_sb = hpool.tile([P, HB, M], mmdt, name="hT")
            w2_sb = w2pool.tile([P, E2B, HB, P], mmdt, name="w2t")

            def mm1_group(w1_t, g, hb, ms):
                ps = psum1.tile([P, ms.stop - ms.start], F32, name="ps1")
                for eb in range(EB):
                    nc.tensor.matmul(
                        ps[:],
                        lhsT=w1_t[:, g, eb, :],
                        rhs=xT_sb[:, eb, ms],
                        start=(eb == 0),
                        stop=(eb == EB - 1),
                    )
                nc.scalar.activation(
                    hT_sb[:, hb, ms], ps[:], act, bias=b1_sb[:, hb : hb + 1]
                )

            # ---- matmul 1 + GELU ----
            # w2 slab prefetches are spread over the mm1 slabs so their
            # transfers ride behind the w1/x stream on the same FIFO.
            # Slab 0 runs m-half-major so x half1 isn't needed until 4
            # groups in (its DMA lands during groups 0-3).
            for s in range(NS1):
                if s < 2:
                    w1_t = w1_tiles[s]
                else:
                    w1_t = w1pool.tile([P, W1G, EB, P], mmdt, name="w1t")
                    nc.sync.dma_start(
                        out=w1_t[:], in_=w1_v[:, s * W1G : (s + 1) * W1G]
                    )
                if s == 0:
                    # m-half-major so x half1 isn't needed until 4 groups in
                    for mh in range(MH):
                        for g in range(W1G):
                            mm1_group(w1_t, g, g, slice(mh * MS, (mh + 1) * MS))
                else:
                    for g in range(W1G):
                        for mh in range(MH):
                            mm1_group(
                                w1_t, g, s * W1G + g,
                                slice(mh * MS, (mh + 1) * MS),
                            )
                if s >= NS1 - E2B:
                    e2b = s - (NS1 - E2B)
                    nc.sync.dma_start(out=w2_sb[:, e2b], in_=w2_d[e2b])

            # ---- matmul 2 + bias ----
            # The last e2b is chunked progressively finer so the final
            # activation + store pipeline drains quickly (shorter tail).
            for e2b in range(E2B):
                out_sb = opool.tile([P, M], mmdt, name="outsb")
                if e2b == E2B - 1 and M % 8 == 0:
                    q = M // 8
                    bounds = [0, 2 * q, 4 * q, 6 * q, 7 * q, 8 * q]
                    chunks = list(zip(bounds[:-1], bounds[1:]))
                else:
                    chunks = [(mh * MS, (mh + 1) * MS) for mh in range(MH)]
                for c0, c1 in chunks:
                    ms = slice(c0, c1)
                    ps2 = psum2.tile([P, c1 - c0], F32, name="ps2")
                    for hb in range(HB):
                        nc.tensor.matmul(
                            ps2[:],
                            lhsT=w2_sb[:, e2b, hb, :],
                            rhs=hT_sb[:, hb, ms],
                            start=(hb == 0),
                            stop=(hb == HB - 1),
                        )
                    nc.scalar.activation(
                        out_sb[:, ms], ps2[:], IDENT, bias=b2_sb[:, e2b : e2b + 1]
                    )
                    nc.sync.dma_start(out=out_d[e2b, :, ms], in_=out_sb[:, ms])

    nc.compile()
    return nc


def pack_inputs(x, w1, b1, w2, b2):
    """Host-side shard + pack (cast matmul operands to bf16). Returns
    per-core input maps."""
    M_TOT = x.shape[0] * x.shape[1]
    E = x.shape[2]
    H = w1.shape[1]
    E2 = w2.shape[1]
    MC = M_TOT // N_CORES
    bf = ml_dtypes.bfloat16
    xf = np.ascontiguousarray(x.reshape(M_TOT, E))

    w1p = np.ascontiguousarray(
        w1.reshape(E // P, P, H // P, P).transpose(2, 1, 0, 3).astype(bf)
    )
    w2p = np.ascontiguousarray(
        w2.reshape(H // P, P, E2 // P, P).transpose(2, 1, 0, 3).astype(bf)
    )
    b1p = np.ascontiguousarray(b1.reshape(H // P, P).T)
    b2p = np.ascontiguousarray(b2.reshape(E2 // P, P).T)

    in_maps = []
    for i in range(N_CORES):
        xTi = np.ascontiguousarray(xf[i * MC : (i + 1) * MC].T.astype(bf))
        in_maps.append(
            {"xT": xTi, "w1p": w1p, "b1p": b1p, "w2p": w2p, "b2p": b2p}
        )
    return in_maps


def unpack_outputs(results, batch_shape=(4, 2048), E2=1024):
    M_TOT = batch_shape[0] * batch_shape[1]
    MC = M_TOT // N_CORES
    out = np.empty((M_TOT, E2), dtype=np.float32)
    for i in range(N_CORES):
        o = results[i]["outT"].astype(np.float32)  # [E2B, P, MC]
        out[i * MC : (i + 1) * MC] = o.transpose(2, 0, 1).reshape(MC, E2)
    return out.reshape(*batch_shape, E2)


_NC_CACHE = {}


def _get_nc():
    if "nc" not in _NC_CACHE:
        _NC_CACHE["nc"] = build_nc()
    return _NC_CACHE["nc"]


def kernel(x, w1, b1, w2, b2):
    nc = _get_nc()
    in_maps = pack_inputs(
        np.asarray(x, dtype=np.float32),
        np.asarray(w1, dtype=np.float32),
        np.asarray(b1, dtype=np.float32),
        np.asarray(w2, dtype=np.float32),
        np.asarray(b2, dtype=np.float32),
    )
    res = run_bass_kernel_spmd(nc, in_maps, core_ids=list(range(N_CORES))).results
    return unpack_outputs(res, batch_shape=(x.shape[0], x.shape[1]), E2=w2.shape[1])


# revision 25
# speedup vs baseline: 1.0306x; 1.0306x over previous
"""Trainium2 Bass kernel for CustomMLP: out = GELU(x@W1+b1)@W2 + b2.

x: (4, 2048, 1024) f32, W1: (1024, 4096), b1: (4096,), W2: (4096, 1024),
b2: (1024,). Data-parallel over the 8192 flattened rows: each of the 8
NeuronCores handles 1024 rows with fully replicated weights (no
collectives).

Matmul operands are bf16 (host-cast); PSUM accumulation is fp32 and both
bias adds run in fp32 on the activation engine, so end-to-end rel err is
~3e-3 (gate 2e-2). bf16 halves DMA traffic and SBUF footprint vs the
fp32r version, which removes the DMA-induced PE stalls.

Per-core layout (everything transposed so both matmuls contract on the
partition axis with no on-chip transposes):
  xT   [1024(e), 1024(m)]           = x_shard^T
  hT   [h, m] computed on chip      (GELU applied on PSUM eviction)
  outT [1024(e2), 1024(m)]          host transposes back

matmul1: psum[h_blk, m] += w1[e_blk, h_blk].T @ xT[e_blk, m]
matmul2: psum[e2_blk, m] += w2[h_blk, e2_blk].T @ hT[h_blk, m]

Perf structure:
- A short warm-up of dummy matmuls on a memset tile runs while the first
  DMAs land, so the PE p-state ramp (0.65->2.4GHz over ~3us of busy time)
  is paid on dummy work instead of real matmuls.
- DMAs are batched (w1 in 4-block slabs, x in halves) to amortize the
  ~600ns/instruction SP sequencer enqueue cost that otherwise delays the
  first transfers.
- w2 is preloaded in full into SBUF during matmul-1 so matmul-2 never
  waits on DMA.
"""
import ml_dtypes
import numpy as np

import concourse.bass as bass
import concourse.mybir as mybir
import concourse.tile as tile
from concourse import bacc
from concourse.bass_utils import run_bass_kernel_spmd

P = 128
N_CORES = 8

F32 = mybir.dt.float32
BF16 = mybir.dt.bfloat16
GELU = mybir.ActivationFunctionType.Gelu
IDENT = mybir.ActivationFunctionType.Identity


def build_nc(M=1024, E=1024, H=4096, E2=1024, mm_dtype=BF16, act=GELU,
             warmup=15):
    """Build + compile the per-core program. M/E/H/E2 parameterized so a
    scaled-down version can run in CoreSim."""
    EB, HB, E2B = E // P, H // P, E2 // P
    MH = max(1, M // 512)  # m halves (moving-dim chunks of <=512)
    MS = M // MH           # moving chunk size
    W1G = min(4, HB)       # w1 h-blocks per DMA slab
    NS1 = HB // W1G        # number of w1 slabs

    mmdt = mm_dtype
    nc = bacc.Bacc(None, target_bir_lowering=False)
    xT_d = nc.declare_dram_parameter("xT", [E, M], mmdt, isOutput=False)
    w1_d = nc.declare_dram_parameter("w1p", [HB, P, EB, P], mmdt, isOutput=False)
    b1_d = nc.declare_dram_parameter("b1p", [P, HB], F32, isOutput=False)
    w2_d = nc.declare_dram_parameter("w2p", [E2B, P, HB, P], mmdt, isOutput=False)
    b2_d = nc.declare_dram_parameter("b2p", [P, E2B], F32, isOutput=False)
    out_d = nc.declare_dram_parameter("outT", [E2B, P, M], mmdt, isOutput=True)

    xT_v = xT_d.rearrange("(eb p) m -> p eb m", p=P)
    w1_v = w1_d.rearrange("h p e q -> p h e q")

    with tile.TileContext(nc) as tc:
        with (
            tc.tile_pool(name="const", bufs=1) as cpool,
            tc.tile_pool(name="xp", bufs=1) as xpool,
            tc.tile_pool(name="hp", bufs=1) as hpool,
            tc.tile_pool(name="w1p", bufs=3) as w1pool,
            tc.tile_pool(name="w2p", bufs=1) as w2pool,
            tc.tile_pool(name="op", bufs=2) as opool,
            tc.tile_pool(name="ps1", bufs=4, space="PSUM") as psum1,
            tc.tile_pool(name="ps2", bufs=3, space="PSUM") as psum2,
            tc.tile_pool(name="psw", bufs=1, space="PSUM") as psumw,
        ):
            # ---- PE warm-up: dummy matmuls on a zeroed tile while the
            # first real DMAs are in flight. Output PSUM is never read.
            if warmup:
                warm_sb = cpool.tile([P, 2 * P], mmdt, name="warm")
                nc.gpsimd.memset(warm_sb[:], 0.0)
                ps_w = psumw.tile([P, 2 * P], F32, name="psw")
                for _ in range(warmup):
                    nc.tensor.matmul(
                        ps_w[:], lhsT=warm_sb[:, 0:P], rhs=warm_sb[:],
                        start=True, stop=True,
                    )

            # ---- DMA FIFO (all on the SP queue — one FIFO doubles as a
            # priority order; parallel queues would let later non-critical
            # transfers steal bandwidth from the critical first tiles):
            # w1 block0, x half0, w1 blocks 1-3, x half1, b1, w1 slab1, b2,
            # then w2 slabs interleaved into the mm1 loop, out stores in mm2.
            w1_tiles = {}
            w1_tiles[0] = w1pool.tile([P, W1G, EB, P], mmdt, name="w1t")
            nc.sync.dma_start(out=w1_tiles[0][:, 0:1], in_=w1_v[:, 0:1])

            xT_sb = xpool.tile([P, EB, M], mmdt, name="xT")
            nc.sync.dma_start(out=xT_sb[:, :, 0:MS], in_=xT_v[:, :, 0:MS])

            if W1G > 1:
                nc.sync.dma_start(out=w1_tiles[0][:, 1:W1G], in_=w1_v[:, 1:W1G])

            for mh in range(1, MH):
                ms = slice(mh * MS, (mh + 1) * MS)
                nc.sync.dma_start(out=xT_sb[:, :, ms], in_=xT_v[:, :, ms])

            b1_sb = cpool.tile([P, HB], F32, name="b1s")
            b2_sb = cpool.tile([P, E2B], F32, name="b2s")
            nc.sync.dma_start(out=b1_sb[:], in_=b1_d[:])

            if NS1 > 1:
                w1_tiles[1] = w1pool.tile([P, W1G, EB, P], mmdt, name="w1t")
                nc.sync.dma_start(out=w1_tiles[1][:], in_=w1_v[:, W1G : 2 * W1G])
            nc.sync.dma_start(out=b2_sb[:], in_=b2_d[:])

            hT_sb = hpool.tile([P, HB, M], mmdt, name="hT")
            w2_sb = w2pool.tile([P, E2B, HB, P], mmdt, name="w2t")

            def mm1_group(w1_t, g, hb, ms):
                ps = psum1.tile([P, ms.stop - ms.start], F32, name="ps1")
                for eb in range(EB):
                    nc.tensor.matmul(
                        ps[:],
                        lhsT=w1_t[:, g, eb, :],
                        rhs=xT_sb[:, eb, ms],
                        start=(eb == 0),
                        stop=(eb == EB - 1),
                    )
                nc.scalar.activation(
                    hT_sb[:, hb, ms], ps[:], act, bias=b1_sb[:, hb : hb + 1]
                )

            # ---- matmul 1 + GELU ----
            # w2 slab prefetches are spread over the mm1 slabs so their
            # transfers ride behind the w1/x stream on the same FIFO.
            # Slab 0 runs m-half-major so x half1 isn't needed until 4
            # groups in (its DMA lands during groups 0-3).
            for s in range(NS1):
                if s < 2:
                    w1_t = w1_tiles[s]
                else:
                    w1_t = w1pool.tile([P, W1G, EB, P], mmdt, name="w1t")
                    nc.sync.dma_start(
                        out=w1_t[:], in_=w1_v[:, s * W1G : (s + 1) * W1G]
                    )
                if s == 0:
                    # m-half-major so x half1 isn't needed until 4 groups in
                    for mh in range(MH):
                        for g in range(W1G):
                            mm1_group(w1_t, g, g, slice(mh * MS, (mh + 1) * MS))
                else:
                    for g in range(W1G):
                        for mh in range(MH):
                            mm1_group(
                                w1_t, g, s * W1G + g,
                                slice(mh * MS, (mh + 1) * MS),
                            )
                if s >= NS1 - E2B:
                    e2b = s - (NS1 - E2B)
                    nc.sync.dma_start(out=w2_sb[:, e2b], in_=w2_d[e2b])

            # ---- matmul 2 + bias ----
            # The last e2b is chunked progressively finer so the final
            # activation + store pipeline drains quickly (shorter tail).
            for e2b in range(E2B):
                out_sb = opool.tile([P, M], mmdt, name="outsb")
                if e2b == E2B - 1 and M % 8 == 0:
                    q = M // 8
                    bounds = [0, 2 * q, 4 * q, 6 * q, 7 * q, 8 * q]
                    chunks = list(zip(bounds[:-1], bounds[1:]))
                else:
                    chunks = [(mh * MS, (mh + 1) * MS) for mh in range(MH)]
                for c0, c1 in chunks:
                    ms = slice(c0, c1)
                    ps2 = psum2.tile([P, c1 - c0], F32, name="ps2")
                    for hb in range(HB):
                        nc.tensor.matmul(
                            ps2[:],
                            lhsT=w2_sb[:, e2b, hb, :],
                            rhs=hT_sb[:, hb, ms],
                            start=(hb == 0),
                            stop=(hb == HB - 1),
                        )
                    nc.scalar.activation(
                        out_sb[:, ms], ps2[:], IDENT, bias=b2_sb[:, e2b : e2b + 1]
                    )
                    nc.sync.dma_start(out=out_d[e2b, :, ms], in_=out_sb[:, ms])

    nc.compile()
    return nc


def pack_inputs(x, w1, b1, w2, b2):
    """Host-side shard + pack (cast matmul operands to bf16). Returns
    per-core input maps."""
    M_TOT = x.shape[0] * x.shape[1]
    E = x.shape[2]
    H = w1.shape[1]
    E2 = w2.shape[1]
    MC = M_TOT // N_CORES
    bf = ml_dtypes.bfloat16
    xf = np.ascontiguousarray(x.reshape(M_TOT, E))

    w1p = np.ascontiguousarray(
        w1.reshape(E // P, P, H // P, P).transpose(2, 1, 0, 3).astype(bf)
    )
    w2p = np.ascontiguousarray(
        w2.reshape(H // P, P, E2 // P, P).transpose(2, 1, 0, 3).astype(bf)
    )
    b1p = np.ascontiguousarray(b1.reshape(H // P, P).T)
    b2p = np.ascontiguousarray(b2.reshape(E2 // P, P).T)

    in_maps = []
    for i in range(N_CORES):
        xTi = np.ascontiguousarray(xf[i * MC : (i + 1) * MC].T.astype(bf))
        in_maps.append(
            {"xT": xTi, "w1p": w1p, "b1p": b1p, "w2p": w2p, "b2p": b2p}
        )
    return in_maps


def unpack_outputs(results, batch_shape=(4, 2048), E2=1024):
    M_TOT = batch_shape[0] * batch_shape[1]
    MC = M_TOT // N_CORES
    out = np.empty((M_TOT, E2), dtype=np.float32)
    for i in range(N_CORES):
        o = results[i]["outT"].astype(np.float32)  # [E2B, P, MC]
        out[i * MC : (i + 1) * MC] = o.transpose(2, 0, 1).reshape(MC, E2)
    return out.reshape(*batch_shape, E2)


_NC_CACHE = {}


def _get_nc():
    if "nc" not in _NC_CACHE:
        _NC_CACHE["nc"] = build_nc()
    return _NC_CACHE["nc"]


def kernel(x, w1, b1, w2, b2):
    nc = _get_nc()
    in_maps = pack_inputs(
        np.asarray(x, dtype=np.float32),
        np.asarray(w1, dtype=np.float32),
        np.asarray(b1, dtype=np.float32),
        np.asarray(w2, dtype=np.float32),
        np.asarray(b2, dtype=np.float32),
    )
    res = run_bass_kernel_spmd(nc, in_maps, core_ids=list(range(N_CORES))).results
    return unpack_outputs(res, batch_shape=(x.shape[0], x.shape[1]), E2=w2.shape[1])


# revision 26
# speedup vs baseline: 1.0327x; 1.0020x over previous
"""Trainium2 Bass kernel for CustomMLP: out = GELU(x@W1+b1)@W2 + b2.

x: (4, 2048, 1024) f32, W1: (1024, 4096), b1: (4096,), W2: (4096, 1024),
b2: (1024,). Data-parallel over the 8192 flattened rows: each of the 8
NeuronCores handles 1024 rows with fully replicated weights (no
collectives).

Matmul operands are bf16 (host-cast); PSUM accumulation is fp32 and both
bias adds run in fp32 on the activation engine, so end-to-end rel err is
~3e-3 (gate 2e-2). bf16 halves DMA traffic and SBUF footprint vs the
fp32r version, which removes the DMA-induced PE stalls.

Per-core layout (everything transposed so both matmuls contract on the
partition axis with no on-chip transposes):
  xT   [1024(e), 1024(m)]           = x_shard^T
  hT   [h, m] computed on chip      (GELU applied on PSUM eviction)
  outT [1024(e2), 1024(m)]          host transposes back

matmul1: psum[h_blk, m] += w1[e_blk, h_blk].T @ xT[e_blk, m]
matmul2: psum[e2_blk, m] += w2[h_blk, e2_blk].T @ hT[h_blk, m]

Perf structure:
- A short warm-up of dummy matmuls on a memset tile runs while the first
  DMAs land, so the PE p-state ramp (0.65->2.4GHz over ~3us of busy time)
  is paid on dummy work instead of real matmuls.
- DMAs are batched (w1 in 4-block slabs, x in halves) to amortize the
  ~600ns/instruction SP sequencer enqueue cost that otherwise delays the
  first transfers.
- w2 is preloaded in full into SBUF during matmul-1 so matmul-2 never
  waits on DMA.
"""
import ml_dtypes
import numpy as np

import concourse.bass as bass
import concourse.mybir as mybir
import concourse.tile as tile
from concourse import bacc
from concourse.bass_utils import run_bass_kernel_spmd

P = 128
N_CORES = 8

F32 = mybir.dt.float32
BF16 = mybir.dt.bfloat16
GELU = mybir.ActivationFunctionType.Gelu
IDENT = mybir.ActivationFunctionType.Identity


def build_nc(M=1024, E=1024, H=4096, E2=1024, mm_dtype=BF16, act=GELU,
             warmup=15):
    """Build + compile the per-core program. M/E/H/E2 parameterized so a
    scaled-down version can run in CoreSim."""
    EB, HB, E2B = E // P, H // P, E2 // P
    MH = max(1, M // 512)  # m halves (moving-dim chunks of <=512)
    MS = M // MH           # moving chunk size
    W1G = min(4, HB)       # w1 h-blocks per DMA slab
    NS1 = HB // W1G        # number of w1 slabs

    mmdt = mm_dtype
    nc = bacc.Bacc(None, target_bir_lowering=False)
    xT_d = nc.declare_dram_parameter("xT", [E, M], mmdt, isOutput=False)
    w1_d = nc.declare_dram_parameter("w1p", [HB, P, EB, P], mmdt, isOutput=False)
    b1_d = nc.declare_dram_parameter("b1p", [P, HB], F32, isOutput=False)
    w2_d = nc.declare_dram_parameter("w2p", [E2B, P, HB, P], mmdt, isOutput=False)
    b2_d = nc.declare_dram_parameter("b2p", [P, E2B], F32, isOutput=False)
    out_d = nc.declare_dram_parameter("outT", [E2B, P, M], mmdt, isOutput=True)

    xT_v = xT_d.rearrange("(eb p) m -> p eb m", p=P)
    w1_v = w1_d.rearrange("h p e q -> p h e q")

    with tile.TileContext(nc) as tc:
        with (
            tc.tile_pool(name="const", bufs=1) as cpool,
            tc.tile_pool(name="xp", bufs=1) as xpool,
            tc.tile_pool(name="hp", bufs=1) as hpool,
            tc.tile_pool(name="w1p", bufs=3) as w1pool,
            tc.tile_pool(name="w2p", bufs=1) as w2pool,
            tc.tile_pool(name="op", bufs=2) as opool,
            tc.tile_pool(name="ps1", bufs=4, space="PSUM") as psum1,
            tc.tile_pool(name="ps2", bufs=3, space="PSUM") as psum2,
            tc.tile_pool(name="psw", bufs=1, space="PSUM") as psumw,
        ):
            # ---- PE warm-up: dummy matmuls on a zeroed tile while the
            # first real DMAs are in flight. Output PSUM is never read.
            if warmup:
                warm_sb = cpool.tile([P, 2 * P], mmdt, name="warm")
                nc.gpsimd.memset(warm_sb[:], 0.0)
                ps_w = psumw.tile([P, 2 * P], F32, name="psw")
                for _ in range(warmup):
                    nc.tensor.matmul(
                        ps_w[:], lhsT=warm_sb[:, 0:P], rhs=warm_sb[:],
                        start=True, stop=True,
                    )

            # ---- DMA FIFO (all on the SP queue — one FIFO doubles as a
            # priority order; parallel queues would let later non-critical
            # transfers steal bandwidth from the critical first tiles):
            # w1 block0, x half0, w1 blocks 1-3, x half1, b1, w1 slab1, b2,
            # then w2 slabs interleaved into the mm1 loop, out stores in mm2.
            w1_tiles = {}
            w1_tiles[0] = w1pool.tile([P, W1G, EB, P], mmdt, name="w1t")
            nc.sync.dma_start(out=w1_tiles[0][:, 0:1], in_=w1_v[:, 0:1])

            xT_sb = xpool.tile([P, EB, M], mmdt, name="xT")
            nc.sync.dma_start(out=xT_sb[:, :, 0:MS], in_=xT_v[:, :, 0:MS])

            if W1G > 1:
                nc.sync.dma_start(out=w1_tiles[0][:, 1:W1G], in_=w1_v[:, 1:W1G])

            for mh in range(1, MH):
                ms = slice(mh * MS, (mh + 1) * MS)
                nc.sync.dma_start(out=xT_sb[:, :, ms], in_=xT_v[:, :, ms])

            b1_sb = cpool.tile([P, HB], F32, name="b1s")
            b2_sb = cpool.tile([P, E2B], F32, name="b2s")
            nc.sync.dma_start(out=b1_sb[:], in_=b1_d[:])

            if NS1 > 1:
                w1_tiles[1] = w1pool.tile([P, W1G, EB, P], mmdt, name="w1t")
                nc.sync.dma_start(out=w1_tiles[1][:], in_=w1_v[:, W1G : 2 * W1G])
            nc.sync.dma_start(out=b2_sb[:], in_=b2_d[:])

            hT_sb = hpool.tile([P, HB, M], mmdt, name="hT")
            w2_sb = w2pool.tile([P, E2B, HB, P], mmdt, name="w2t")

            def mm1_group(w1_t, g, hb, ms):
                ps = psum1.tile([P, ms.stop - ms.start], F32, name="ps1")
                for eb in range(EB):
                    nc.tensor.matmul(
                        ps[:],
                        lhsT=w1_t[:, g, eb, :],
                        rhs=xT_sb[:, eb, ms],
                        start=(eb == 0),
                        stop=(eb == EB - 1),
                    )
                nc.scalar.activation(
                    hT_sb[:, hb, ms], ps[:], act, bias=b1_sb[:, hb : hb + 1]
                )

            # ---- matmul 1 + GELU ----
            # w2 slab prefetches are spread over the mm1 slabs so their
            # transfers ride behind the w1/x stream on the same FIFO.
            # Slab 0 runs m-half-major so x half1 isn't needed until 4
            # groups in (its DMA lands during groups 0-3).
            for s in range(NS1):
                if s < 2:
                    w1_t = w1_tiles[s]
                else:
                    w1_t = w1pool.tile([P, W1G, EB, P], mmdt, name="w1t")
                    nc.sync.dma_start(
                        out=w1_t[:], in_=w1_v[:, s * W1G : (s + 1) * W1G]
                    )
                if s == 0:
                    # m-half-major so x half1 isn't needed until 4 groups in
                    for mh in range(MH):
                        for g in range(W1G):
                            mm1_group(w1_t, g, g, slice(mh * MS, (mh + 1) * MS))
                else:
                    for g in range(W1G):
                        for mh in range(MH):
                            mm1_group(
                                w1_t, g, s * W1G + g,
                                slice(mh * MS, (mh + 1) * MS),
                            )
                if s >= NS1 - E2B:
                    e2b = s - (NS1 - E2B)
                    nc.sync.dma_start(out=w2_sb[:, e2b], in_=w2_d[e2b])

            # ---- matmul 2 + bias ----
            # The last e2b is chunked progressively finer so the final
            # activation + store pipeline drains quickly (shorter tail).
            for e2b in range(E2B):
                out_sb = opool.tile([P, M], mmdt, name="outsb")
                if e2b == E2B - 1 and M % 8 == 0:
                    q = M // 8
                    bounds = [0, 2 * q, 4 * q, 6 * q, 7 * q, 8 * q]
                    chunks = list(zip(bounds[:-1], bounds[1:]))
                else:
                    chunks = [(mh * MS, (mh + 1) * MS) for mh in range(MH)]
                for c0, c1 in chunks:
                    ms = slice(c0, c1)
                    ps2 = psum2.tile([P, c1 - c0], F32, name="ps2")
                    for hb in range(HB):
                        nc.tensor.matmul(
                            ps2[:],
                            lhsT=w2_sb[:, e2b, hb, :],
                            rhs=hT_sb[:, hb, ms],
                            start=(hb == 0),
                            stop=(hb == HB - 1),
                        )
                    nc.scalar.activation(
                        out_sb[:, ms], ps2[:], IDENT, bias=b2_sb[:, e2b : e2b + 1]
                    )
                    # store from the ACT engine's own HWDGE queue: the
                    # enqueue follows the activation on the same engine,
                    # skipping a cross-engine semaphore hop on the tail
                    nc.scalar.dma_start(out=out_d[e2b, :, ms], in_=out_sb[:, ms])

    nc.compile()
    return nc


def pack_inputs(x, w1, b1, w2, b2):
    """Host-side shard + pack (cast matmul operands to bf16). Returns
    per-core input maps."""
    M_TOT = x.shape[0] * x.shape[1]
    E = x.shape[2]
    H = w1.shape[1]
    E2 = w2.shape[1]
    MC = M_TOT // N_CORES
    bf = ml_dtypes.bfloat16
    xf = np.ascontiguousarray(x.reshape(M_TOT, E))

    w1p = np.ascontiguousarray(
        w1.reshape(E // P, P, H // P, P).transpose(2, 1, 0, 3).astype(bf)
    )
    w2p = np.ascontiguousarray(
        w2.reshape(H // P, P, E2 // P, P).transpose(2, 1, 0, 3).astype(bf)
    )
    b1p = np.ascontiguousarray(b1.reshape(H // P, P).T)
    b2p = np.ascontiguousarray(b2.reshape(E2 // P, P).T)

    in_maps = []
    for i in range(N_CORES):
        xTi = np.ascontiguousarray(xf[i * MC : (i + 1) * MC].T.astype(bf))
        in_maps.append(
            {"xT": xTi, "w1p": w1p, "b1p": b1p, "w2p": w2p, "b2p": b2p}
        )
    return in_maps


def unpack_outputs(results, batch_shape=(4, 2048), E2=1024):
    M_TOT = batch_shape[0] * batch_shape[1]
    MC = M_TOT // N_CORES
    out = np.empty((M_TOT, E2), dtype=np.float32)
    for i in range(N_CORES):
        o = results[i]["outT"].astype(np.float32)  # [E2B, P, MC]
        out[i * MC : (i + 1) * MC] = o.transpose(2, 0, 1).reshape(MC, E2)
    return out.reshape(*batch_shape, E2)


_NC_CACHE = {}


def _get_nc():
    if "nc" not in _NC_CACHE:
        _NC_CACHE["nc"] = build_nc()
    return _NC_CACHE["nc"]


def kernel(x, w1, b1, w2, b2):
    nc = _get_nc()
    in_maps = pack_inputs(
        np.asarray(x, dtype=np.float32),
        np.asarray(w1, dtype=np.float32),
        np.asarray(b1, dtype=np.float32),
        np.asarray(w2, dtype=np.float32),
        np.asarray(b2, dtype=np.float32),
    )
    res = run_bass_kernel_spmd(nc, in_maps, core_ids=list(range(N_CORES))).results
    return unpack_outputs(res, batch_shape=(x.shape[0], x.shape[1]), E2=w2.shape[1])
